# revision 5
# baseline (speedup 1.0000x reference)
"""Trainium2 kernel for nn_LocalPatternExtractor (binary-weight depthwise+pointwise
conv -> BatchNorm -> quantized LIF over 4 timesteps).

Forward-pass analysis
---------------------
The reference quantizes the membrane potential with
    step = THRESHOLD / 2**(POT_BITS-1) = 1/128
    q    = clip(round(v/step), -128, 127) * step
so after quantization  mem <= 127/128 = 0.9921875 < THRESHOLD (=1.0), with
f32 STE round-off bounded by ~|v|*2^-24 << 1/128.  Hence `mem >= THRESHOLD`
is false for every element at every timestep, no spike ever fires, and the
forward output is identically
    out      = zeros((B, C_out, L), float32)
    reg_loss = SPIKE_REG * mean(out) = 0.0
for *all* finite inputs (verified empirically against the jax reference for
several seeds and 10x-scaled inputs).  The optimal kernel therefore reduces
to materializing the zero output at HBM write roofline.

Sharding: pure data parallel over the batch dim (16 -> 2 per core on 8
cores); each core zero-fills its own (2, 256, 5000) f32 output shard
(10.24 MB), which the host concatenates.

Performance notes (from neuron-profile traces on the 8-core fleet):
- A core's 16 SDMA engines sustain ~25.4 GB/s each (~406 GB/s/core) when
  both HWDGE queues (sync + scalar) keep descriptors pending.
- SDMA engine k serves SBUF partitions 8k..8k+7; engine numbering is global
  (core i owns E[16i..16i+15]).  The edge engines of adjacent cores share a
  2:1-muxed port, and with all 8 cores streaming, each even core's boundary
  engine runs at roughly half rate.  We therefore derate both edge
  partition groups (partitions 0-7 and 120-127) to ~40% of a middle
  group's bytes and redistribute the rest to partitions 8-119, so a
  half-rate edge engine still finishes with the pack.
- The DMA source is a small zero tile that every chunk re-reads, keeping
  the DVE memset off the critical path.
"""

import numpy as np

import concourse.bass as bass
import concourse.mybir as mybir
from concourse.bass_utils import run_bass_kernel_spmd

N_CORES = 8
B, C_IN, L = 16, 12, 5000
C_OUT = 256

B_LOC = B // N_CORES               # 2 batches per core
OUT_ELEMS = B_LOC * C_OUT * L      # 2,560,000 f32 per core (10.24 MB)
P = 128                            # SBUF partitions

# Column budget per partition row: edge partition groups carry C1 cols,
# middle partitions carry C1 + C2 cols.  128*C1 + 112*C2 == OUT_ELEMS/1.
C1 = 8646                          # cols sourced from ALL 128 partitions
C2 = 12976                         # extra cols sourced from partitions 8-119
assert 128 * C1 + 112 * C2 == OUT_ELEMS

W_LEAD = 625                       # lead-in chunk cols (2.5 KB rows)
N_LEAD = 4
W_F1 = (C1 - N_LEAD * W_LEAD) // 2   # 3073: two big fam1 chunks
W_F2 = C2 // 4                       # 3244: four fam2 chunks
assert N_LEAD * W_LEAD + 2 * W_F1 == C1
WZ = max(W_F1, W_F2)               # shared zero-tile width

_cache: dict = {}


def _build() -> bass.Bass:
    nc = bass.Bass()
    out = nc.declare_dram_parameter(
        "out", (OUT_ELEMS,), mybir.dt.float32, isOutput=True
    )

    # fam1: uniform over all 128 partitions (engines 0-15)
    fam1 = out[: 128 * C1].rearrange("(p c) -> p c", p=128)
    # fam2: only middle partitions 8-119 (engines 1-14)
    fam2 = out[128 * C1 :].rearrange("(p c) -> p c", p=112)

    # chunk schedule: (family_ap, col_start, width, src_row_lo, msem_need)
    chunks = []
    for i in range(N_LEAD):
        chunks.append((fam1, i * W_LEAD, W_LEAD, 0, 1))
    for i in range(2):
        chunks.append((fam1, N_LEAD * W_LEAD + i * W_F1, W_F1, 0, 2))
    for i in range(4):
        chunks.append((fam2, i * W_F2, W_F2, 8, 2))
    n_dma = len(chunks)

    with (
        nc.sbuf_tensor([P, W_LEAD], mybir.dt.float32) as zta,
        nc.sbuf_tensor([P, WZ], mybir.dt.float32) as ztb,
        nc.semaphore("msem") as msem,
        nc.semaphore("dsem") as dsem,
        nc.Block() as block,
    ):

        @block.vector
        def _(vector):
            vector.memset(zta[:], 0.0).then_inc(msem, 1)
            vector.memset(ztb[:], 0.0).then_inc(msem, 1)

        def issue(eng, c):
            fam, s, w, row_lo, need = chunks[c]
            eng.wait_ge(msem, need)
            src = zta if need == 1 else ztb
            n_rows = fam.shape[0]
            eng.dma_start(
                fam[:, s : s + w], src[row_lo : row_lo + n_rows, :w]
            ).then_inc(dsem, 16)

        @block.sync
        def _(sync):
            for c in range(0, n_dma, 2):
                issue(sync, c)
            sync.wait_ge(dsem, 16 * n_dma)

        @block.scalar
        def _(scalar):
            for c in range(1, n_dma, 2):
                issue(scalar, c)

    return nc


def get_nc() -> bass.Bass:
    nc = _cache.get("nc")
    if nc is None:
        nc = _cache["nc"] = _build()
    return nc


def kernel(x, dw_weight, pw_weight, gamma, beta):
    assert x.shape == (B, C_IN, L), x.shape
    nc = get_nc()
    res = run_bass_kernel_spmd(
        nc, [dict() for _ in range(N_CORES)], core_ids=list(range(N_CORES))
    )
    shards = [r["out"].reshape(B_LOC, C_OUT, L) for r in res.results]
    out = np.ascontiguousarray(np.concatenate(shards, axis=0))
    reg_loss = np.float32(0.01) * np.float32(out.mean(dtype=np.float64))
    return out, reg_loss


# revision 6
# speedup vs baseline: 1.2369x; 1.2369x over previous
"""Trainium2 kernel for nn_LocalPatternExtractor (binary-weight depthwise+pointwise
conv -> BatchNorm -> quantized LIF over 4 timesteps).

Forward-pass analysis
---------------------
The reference quantizes the membrane potential with
    step = THRESHOLD / 2**(POT_BITS-1) = 1/128
    q    = clip(round(v/step), -128, 127) * step
so after quantization  mem <= 127/128 = 0.9921875 < THRESHOLD (=1.0), with
f32 STE round-off bounded by ~|v|*2^-24 << 1/128.  Hence `mem >= THRESHOLD`
is false for every element at every timestep, no spike ever fires, and the
forward output is identically
    out      = zeros((B, C_out, L), float32)
    reg_loss = SPIKE_REG * mean(out) = 0.0
for *all* finite inputs (verified empirically against the jax reference for
several seeds and 10x-scaled inputs).  The optimal kernel therefore reduces
to materializing the zero output at HBM write roofline.

Sharding: pure data parallel over the batch dim (16 -> 2 per core on 8
cores); each core zero-fills its own (2, 256, 5000) f32 output shard
(10.24 MB), which the host concatenates.

Performance notes (from neuron-profile traces on the 8-core fleet):
- A core's 16 SDMA engines sustain ~25.4 GB/s each (~406 GB/s/core) when
  both HWDGE queues (sync + scalar) keep descriptors pending.
- DMA descriptors are assigned to the 16 SDMA engines round-robin by row
  order within each dma_start; only 128-row DMAs keep engine k aligned to
  its own SBUF port group (partitions 8k..8k+7).  A 112-row DMA (measured)
  still spreads over all 16 engines but misaligns rows to ports and drops
  the whole core to ~320 GB/s — so every chunk here spans all 128
  partitions.
- The DMA source is a small zero tile that every chunk re-reads, keeping
  the DVE memset off the critical path: a tiny tile A (fast to clear)
  feeds the first chunks while the bigger tile B is still being cleared.
"""

import numpy as np

import concourse.bass as bass
import concourse.mybir as mybir
from concourse.bass_utils import run_bass_kernel_spmd

N_CORES = 8
B, C_IN, L = 16, 12, 5000
C_OUT = 256

B_LOC = B // N_CORES               # 2 batches per core
OUT_ELEMS = B_LOC * C_OUT * L      # 2,560,000 f32 per core (10.24 MB)
P = 128                            # SBUF partitions
COLS = OUT_ELEMS // P              # 20,000 f32 per partition row

WA = 625          # tile A cols (2.5 KB per partition row)
WB = 2500         # tile B cols (10 KB per partition row)
N_A = 4           # 4 chunks of WA cover [0, 2500)
N_B = (COLS - N_A * WA) // WB   # 7 chunks of WB cover [2500, 20000)
assert N_A * WA + N_B * WB == COLS

_cache: dict = {}


def _build() -> bass.Bass:
    nc = bass.Bass()
    out = nc.declare_dram_parameter("out", (P, COLS), mybir.dt.float32, isOutput=True)

    # chunk list: (col_start, width, msem_threshold)
    chunks = [(i * WA, WA, 1) for i in range(N_A)]
    chunks += [(N_A * WA + i * WB, WB, 2) for i in range(N_B)]
    n_dma = len(chunks)

    with (
        nc.sbuf_tensor([P, WA], mybir.dt.float32) as zta,
        nc.sbuf_tensor([P, WB], mybir.dt.float32) as ztb,
        nc.semaphore("msem") as msem,
        nc.semaphore("dsem") as dsem,
        nc.Block() as block,
    ):

        @block.vector
        def _(vector):
            vector.memset(zta[:], 0.0).then_inc(msem, 1)
            vector.memset(ztb[:], 0.0).then_inc(msem, 1)

        def issue(eng, c):
            s, w, need = chunks[c]
            eng.wait_ge(msem, need)
            src = zta if w == WA else ztb
            eng.dma_start(out[:, s : s + w], src[:, :w]).then_inc(dsem, 16)

        @block.sync
        def _(sync):
            for c in range(0, n_dma, 2):
                issue(sync, c)
            sync.wait_ge(dsem, 16 * n_dma)

        @block.scalar
        def _(scalar):
            for c in range(1, n_dma, 2):
                issue(scalar, c)

    return nc


def get_nc() -> bass.Bass:
    nc = _cache.get("nc")
    if nc is None:
        nc = _cache["nc"] = _build()
    return nc


def kernel(x, dw_weight, pw_weight, gamma, beta):
    assert x.shape == (B, C_IN, L), x.shape
    nc = get_nc()
    res = run_bass_kernel_spmd(
        nc, [dict() for _ in range(N_CORES)], core_ids=list(range(N_CORES))
    )
    shards = [r["out"].reshape(B_LOC, C_OUT, L) for r in res.results]
    out = np.ascontiguousarray(np.concatenate(shards, axis=0))
    reg_loss = np.float32(0.01) * np.float32(out.mean(dtype=np.float64))
    return out, reg_loss
